# revision 24
# baseline (speedup 1.0000x reference)
"""Trainium2 Bass kernel for nn_CHAN_29764123361280 (ragged_sequence).

Sharding: data-parallel over the batch axis B=8 -> one video per NeuronCore,
all weights replicated. Per-core pipeline (32 segments):
  conv1(k5,p2)+maxpool2 -> conv2(k5,p2)+maxpool2 -> additive self-attention
  + two concept attentions -> concat -> deconv1 -> deconv2 -> similarity
  scoring.  All matmuls bf16/fp8 with fp32 PSUM accumulation.

The additive-attention tanh is replaced by an odd cubic polynomial
(tanh(z) ~ c1 z + c3 z^3, weighted-LSQ fit on |z|<=5.2).  Expanding
(qp+kp)^3 turns the score tensor into a sum of rank-256 bilinear forms:
  s[q,k] = <v o qp^2, kp>*3c3 + <v o qp, kp^2>*3c3 + <v, c1 kp + c3 kp^3>
(pure-q terms are softmax-invariant and dropped), i.e. six N=128 matmuls
per 4-segment group instead of the L4^2*H elementwise tanh monster.
End-to-end rel-err of this approximation is ~1.8e-4, far below the fp8
conv noise (~1.6e-3).

Scores for 4 segments land in one PSUM tile [(4s,q)=128, (4s,k)=128];
cross-segment blocks are killed by a block-diagonal mask before the
softmax, so one PE transpose of the softmaxed tile directly yields the
block-diagonal rhs for a batched self-attention-result matmul.

The conv stage for supergroup sg+1 is emitted between the projection and
attention phases of supergroup sg so the PE has conv work while the
attention chain runs.
"""

from contextlib import ExitStack

import numpy as np
import ml_dtypes

import concourse.bass as bass  # noqa: F401
import concourse.mybir as mybir
import concourse.tile as tile
from concourse import bacc
from concourse.bass_utils import run_bass_kernel_spmd
from concourse.masks import make_identity

BF16 = mybir.dt.bfloat16
F32 = mybir.dt.float32
FP8 = mybir.dt.float8e4
DR = mybir.MatmulPerfMode.DoubleRow
SC = 256.0           # fp8 weight pre-scale (undone via activation scale)
ISC = 1.0 / SC
nf8 = ml_dtypes.float8_e4m3

B, S, L, CIN = 8, 32, 128, 1024
C1, C2 = 512, 256
D1, D2 = 512, 128
CD, SIM = 300, 128
L4 = L // 4          # 32
SEG = S              # segments per core
NEG = -30.0          # mask logit bias (exp(-30) ~ 1e-13)

nbf = ml_dtypes.bfloat16

DC_TAPS = ((0, ((1, 1), (3, 0))), (1, ((2, 1), (0, 2))))
# parity -> ((tap, input col offset), ...) for ConvTranspose1d(k=4,s=2,p=1)
# on halo'd input xh[:, u+1] = x[:, u]:
#   even out j=2u:  W1.x[u]  + W3.x[u-1]
#   odd  out j=2u+1: W2.x[u] + W0.x[u+1]

# odd-cubic tanh fit (weighted LSQ on |z|<=5.2; z=qp+kp stays within ~4.7)
PC1 = 0.63982097
PC3 = -0.02262075
PE3 = 3.0 * PC3          # coefficient of the q^2 k / q k^2 cross terms

AX = mybir.AxisListType
OP = mybir.AluOpType
ACTF = mybir.ActivationFunctionType


class _Env:
    pass


def _setup(ctx, tc, io):
    nc = tc.nc
    e = _Env()
    e.nc, e.tc, e.io = nc, tc, io
    singles = ctx.enter_context(tc.tile_pool(name="singles", bufs=1))
    e.singles = singles
    # ---- resident weights / constants ----
    # conv/deconv weights fp8 (x256), paired on dim1+2 for DoubleRow
    e.c1w8 = singles.tile([128, 4, 2, 5, C1], FP8)
    e.c2w8 = singles.tile([128, 2, 2, 5, C2], FP8)
    e.dc1w8 = singles.tile([128, 4, 2, 4, D1], FP8)
    e.dc2w8 = singles.tile([128, 2, 2, 4, D2], FP8)
    e.saq = singles.tile([128, 2, C2], BF16)
    e.sak = singles.tile([128, 2, C2], BF16)
    e.cak = singles.tile([128, 2, C2], BF16)
    e.caq = singles.tile([128, 3, C2], BF16)
    e.s1w = singles.tile([128, SIM], BF16)
    e.s2w = singles.tile([128, 3, SIM], BF16)
    e.c1b = singles.tile([128, 4], F32)
    e.c2b = singles.tile([128, 2], F32)
    e.dc1b = singles.tile([128, 4], F32)
    e.dc2b = singles.tile([128, 1], F32)
    e.sbqk = singles.tile([128, 2], F32)
    e.cab = singles.tile([128, 2], F32)
    e.mlpw = singles.tile([128, 1], F32)
    e.mlpb = singles.tile([16, 1], F32)
    e.cvec = singles.tile([128, 3, 2], BF16)
    e.ve21 = singles.tile([128, 2], F32)    # 3*c3 * sa_v   (h chunks)
    e.ve12 = singles.tile([128, 2], F32)
    e.vch = singles.tile([128, 2], F32)     # sa_v
    e.cve21 = singles.tile([128, 2], F32)   # 3*c3 * ca_v
    e.cve12 = singles.tile([128, 2], F32)
    e.cavch = singles.tile([128, 2], F32)   # ca_v
    e.mb4 = singles.tile([128, 4, 2, 128], F32)  # [(4s,q), sg, g, (4s,k)]
    e.mc4 = singles.tile([2, 4, 2, 128], F32)    # [ci, sg, g, (4s,k)]
    e.blk = singles.tile([128, 4, 2], BF16)  # 0/1 seg-block pattern
    e.ones = singles.tile([128, 128], BF16)
    e.ident = singles.tile([128, 128], BF16)

    # ---- pools ----
    e.xp = ctx.enter_context(tc.tile_pool(name="xp", bufs=3))
    e.t1p = ctx.enter_context(tc.tile_pool(name="t1p", bufs=3))
    e.t2p = ctx.enter_context(tc.tile_pool(name="t2p", bufs=2))
    e.atp = ctx.enter_context(tc.tile_pool(name="atp", bufs=2))
    e.powp = ctx.enter_context(tc.tile_pool(name="powp", bufs=2))
    e.arp = ctx.enter_context(tc.tile_pool(name="arp", bufs=2))
    e.r1p = ctx.enter_context(tc.tile_pool(name="r1p", bufs=2))
    e.r2p = ctx.enter_context(tc.tile_pool(name="r2p", bufs=2))
    e.smp = ctx.enter_context(tc.tile_pool(name="smp", bufs=3))
    e.php = ctx.enter_context(tc.tile_pool(name="php", bufs=1))
    e.finp = ctx.enter_context(tc.tile_pool(name="finp", bufs=2))

    # PSUM: conv/work(2x1) + s(1) + sc(1) + transp(2x1) + score/sim(1)
    e.wps = ctx.enter_context(tc.tile_pool(name="wps", bufs=2, space="PSUM"))
    e.sps = ctx.enter_context(tc.tile_pool(name="sps", bufs=1, space="PSUM"))
    e.tps = ctx.enter_context(tc.tile_pool(name="tps", bufs=1, space="PSUM"))
    e.scp = ctx.enter_context(tc.tile_pool(name="scp", bufs=1, space="PSUM"))

    e.xg0 = []
    for g in range(2):
        xg = e.xp.tile([128, 4, 2, 132, 4], FP8, name=f"xg0{g}", tag="xg")
        e.xg0.append(xg)
    # startup-critical DMAs on separate engine queues so the transfers
    # overlap instead of serializing on one queue
    nc.sync.dma_start(out=e.xg0[0][:], in_=io["x"][0])
    nc.scalar.dma_start(out=e.c1w8[:, 0], in_=io["c1w"][:, 0])
    nc.gpsimd.dma_start(out=e.c1w8[:, 1], in_=io["c1w"][:, 1])
    nc.scalar.dma_start(out=e.c1w8[:, 2], in_=io["c1w"][:, 2])
    nc.gpsimd.dma_start(out=e.c1w8[:, 3], in_=io["c1w"][:, 3])
    nc.sync.dma_start(out=e.xg0[1][:], in_=io["x"][1])
    for t_sb, name in [
        (e.cvec, "cvec"), (e.caq, "caq"), (e.s2w, "s2w"),
        (e.c1b, "c1b"), (e.c2b, "c2b"), (e.dc1b, "dc1b"), (e.dc2b, "dc2b"),
        (e.sbqk, "sbqk"), (e.cab, "cab"), (e.mlpw, "mlpw"),
        (e.mlpb, "mlpb"), (e.mb4, "mb4"), (e.mc4, "mc4"),
        (e.ve21, "ve21"), (e.ve12, "ve12"), (e.vch, "vch"),
        (e.cve21, "cve21"), (e.cve12, "cve12"), (e.cavch, "cavch"),
        (e.blk, "blk"),
    ]:
        nc.sync.dma_start(out=t_sb[:], in_=io[name])
    nc.sync.dma_start(out=e.c2w8[:], in_=io["c2w"])
    nc.sync.dma_start(out=e.saq[:], in_=io["saq"])
    nc.sync.dma_start(out=e.sak[:], in_=io["sak"])
    nc.sync.dma_start(out=e.cak[:], in_=io["cak"])
    make_identity(nc, e.ident[:])
    nc.vector.memset(e.ones[:], 1.0)

    # Touch every DMA'd tensor an engine will read, one instruction per
    # tensor, so each engine's vector clock observes the DMA semaphores
    # early: later compute then never needs >1 sync wait per instruction
    # (the walrus TT/STT encodings only carry one).
    e.dve_scr = singles.tile([1, 16], F32)
    e.act_scr = singles.tile([1, 16], F32)
    for i, t_sb in enumerate(
            (e.cab, e.mlpw, e.mb4[:, 0, 0, :], e.dc1b, e.dc2b, e.sbqk,
             e.c1b, e.c2b, e.mlpb, e.mc4[:, 0, 0, :], e.ve21, e.ve12,
             e.vch, e.blk[:, 0, :])):
        nc.vector.tensor_copy(out=e.dve_scr[0:1, i : i + 1],
                              in_=t_sb[0:1, 0:1])
        nc.scalar.copy(out=e.act_scr[0:1, i : i + 1], in_=t_sb[0:1, 0:1])

    return e


def _phase0(e):
    nc = e.nc
    e.cA = e.php.tile([128, 2, 3, 2], BF16)    # concept lhsT stacks
    e.uband = e.php.tile([128, 512], BF16)     # banded (ci-interleaved) u
    cqb = e.php.tile([128, 2, 2], F32)
    cq2 = e.php.tile([128, 2, 2], F32)
    cq_ps = e.wps.tile([128, 2, 2], F32, tag="work")
    for i, (ci, ht) in enumerate([(c, h) for c in range(2) for h in range(2)]):
        for kc in range(3):
            nc.tensor.matmul(
                cq_ps[:, ht, ci : ci + 1],
                e.caq[:, kc, ht * 128 : (ht + 1) * 128],
                e.cvec[:, kc, ci : ci + 1],
                start=(i == 0 and kc == 0), stop=(i == 3 and kc == 2))
    for ci in range(2):
        for ht in range(2):
            nc.vector.tensor_tensor(
                out=cqb[:, ht, ci : ci + 1], in0=cq_ps[:, ht, ci : ci + 1],
                in1=e.cab[:, ht : ht + 1], op=OP.add)
    nc.vector.tensor_tensor(out=cq2[:], in0=cqb[:], in1=cqb[:], op=OP.mult)
    for ht in range(2):
        nc.vector.tensor_scalar_mul(
            out=e.cA[:, ht, 0, :], in0=cq2[:, ht, :],
            scalar1=e.cve21[:, ht : ht + 1])
        nc.vector.tensor_scalar_mul(
            out=e.cA[:, ht, 1, :], in0=cqb[:, ht, :],
            scalar1=e.cve12[:, ht : ht + 1])
        nc.vector.tensor_copy(
            out=e.cA[:, ht, 2, :],
            in_=e.cavch[:, ht : ht + 1].broadcast_to([128, 2]))
    cw_ps = e.wps.tile([128, 2], F32, tag="work")
    for ci in range(2):
        for kc in range(3):
            nc.tensor.matmul(
                cw_ps[:, ci : ci + 1], e.s2w[:, kc, :],
                e.cvec[:, kc, ci : ci + 1],
                start=(ci == 0 and kc == 0), stop=(ci == 1 and kc == 2))
    nc.vector.memset(e.uband[:], 0.0)
    for ci in range(2):
        nc.vector.tensor_tensor(
            out=e.uband[:, 256 + ci : 257 + ci], in0=cw_ps[:, ci : ci + 1],
            in1=e.mlpw[:, 0:1], op=OP.mult)
    nc.scalar.copy(out=e.act_scr[0:1, 15:16], in_=e.cA[0:1, 0, 0, 0:1])
    e.tc.no_sync_barrier()


def _conv_stage(e, sg, xgs=None):
    """conv1+pool+conv2+pool for supergroup sg -> tmp2 [128, 2, 8, 32]."""
    nc = e.nc
    tmp2 = e.t2p.tile([128, 2, 8, L4], BF16, tag="tmp2")
    for g in range(2):
        if xgs is not None:
            xg = xgs[g]
        else:
            xg = e.xp.tile([128, 4, 2, 132, 4], FP8, tag="xg")
            nc.sync.dma_start(out=xg[:], in_=e.io["x"][sg * 2 + g])

        # seg-minor layouts: free cols = (pos, seg) so the DoubleRow
        # moving AP stays 3-dim [K, 2, N]
        t1 = e.t1p.tile([128, 2, 2, 68, 4], FP8, tag="t1")
        nc.vector.memset(t1[:, :, :, 0:2, :], 0.0)
        nc.vector.memset(t1[:, :, :, 66:68, :], 0.0)
        for m in range(4):
            y1 = e.wps.tile([128, 128, 4], F32, tag="conv")
            n_mm = 0
            for kcp in range(4):
                for t in range(5):
                    for h in range(2):     # same weights back-to-back
                        nc.tensor.matmul(
                            y1[:, h * 64 : (h + 1) * 64, :],
                            e.c1w8[:, kcp, :, t, m * 128 : (m + 1) * 128],
                            xg[:, kcp, :, t + h * 64 : t + h * 64 + 64, :],
                            start=(n_mm == 0), stop=(n_mm == 39),
                            perf_mode=DR)
                        n_mm += 1
            ys = e.t1p.tile([128, 128, 4], F32, tag="pool1")
            nc.scalar.activation(out=ys[:], in_=y1[:], func=ACTF.Identity,
                                 bias=e.c1b[:, m : m + 1], scale=ISC)
            yv = ys[:].rearrange("p (u two) s -> p u two s", two=2)
            nc.vector.tensor_tensor(
                out=t1[:, m // 2, m % 2, 2:66, :], in0=yv[:, :, 0, :],
                in1=yv[:, :, 1, :], op=OP.max)

        for m in range(2):
            y2 = e.wps.tile([128, 64, 4], F32, tag="conv")
            n_mm = 0
            for kcp in range(2):
                for t in range(5):
                    nc.tensor.matmul(
                        y2[:], e.c2w8[:, kcp, :, t, m * 128 : (m + 1) * 128],
                        t1[:, kcp, :, t : t + 64, :],
                        start=(n_mm == 0), stop=(n_mm == 9),
                        perf_mode=DR)
                    n_mm += 1
            ys = e.t1p.tile([128, 64, 4], F32, tag="pool2")
            nc.scalar.activation(out=ys[:], in_=y2[:], func=ACTF.Identity,
                                 bias=e.c2b[:, m : m + 1], scale=ISC)
            yv = ys[:].rearrange("p (u two) s -> p u two s", two=2)
            nc.vector.tensor_tensor(
                out=tmp2[:, m, g * 4 : g * 4 + 4, :],
                in0=yv[:, :, 0, :].rearrange("p u s -> p s u"),
                in1=yv[:, :, 1, :].rearrange("p u s -> p s u"), op=OP.max)
    return tmp2


def _proj_stage(e, tmp2):
    """qp/kp/ck projections (bf16 SBUF) + polynomial power tiles."""
    nc = e.nc
    qk_sb = e.atp.tile([128, 2, 2, 8, L4], BF16, tag="qk")  # [q/k, ht, s, q]
    ck_sb = e.atp.tile([128, 2, 8, L4], BF16, tag="ck")     # [ht, s, k]
    for ht in range(2):
        qp_ps = e.wps.tile([128, 8, L4], F32, tag="conv")
        for kc in range(2):
            nc.tensor.matmul(
                qp_ps[:], e.saq[:, kc, ht * 128 : (ht + 1) * 128],
                tmp2[:, kc, :, :], start=(kc == 0), stop=(kc == 1))
        nc.vector.tensor_scalar_add(
            out=qk_sb[:, 0, ht, :, :], in0=qp_ps[:],
            scalar1=e.sbqk[:, ht : ht + 1])
        kp_ps = e.wps.tile([128, 8, L4], F32, tag="conv")
        for kc in range(2):
            nc.tensor.matmul(
                kp_ps[:], e.sak[:, kc, ht * 128 : (ht + 1) * 128],
                tmp2[:, kc, :, :], start=(kc == 0), stop=(kc == 1))
        nc.vector.tensor_copy(out=qk_sb[:, 1, ht, :, :], in_=kp_ps[:])
        ck = e.wps.tile([128, 8, L4], F32, tag="work")
        for kc in range(2):
            nc.tensor.matmul(
                ck[:], e.cak[:, kc, ht * 128 : (ht + 1) * 128],
                tmp2[:, kc, :, :], start=(kc == 0), stop=(kc == 1))
        nc.vector.tensor_copy(out=ck_sb[:, ht, :, :], in_=ck[:])

    # polynomial stacks (bf16):
    #   lhsT chunks: vq2e = 3c3*(v o qp^2); vqe = 3c3*(v o qp); ones
    #   rhs  chunks: kp (raw); k2 = kp^2; vcombo = v o (c1 kp + c3 kp^3)
    pw = e.powp
    vq2e = pw.tile([128, 2, 8, L4], BF16, tag="vq2e")
    vqe = pw.tile([128, 2, 8, L4], BF16, tag="vqe")
    k2 = pw.tile([128, 2, 8, L4], BF16, tag="k2")
    vcombo = pw.tile([128, 2, 8, L4], BF16, tag="vcombo")
    ck2 = pw.tile([128, 2, 8, L4], BF16, tag="ck2")
    cvcombo = pw.tile([128, 2, 8, L4], BF16, tag="cvcombo")
    scr = pw.tile([128, 2, 8, L4], BF16, tag="pscr")
    scr2 = pw.tile([128, 2, 8, L4], BF16, tag="pscr2")
    nc.vector.tensor_tensor(
        out=scr[:], in0=qk_sb[:, 0], in1=qk_sb[:, 0], op=OP.mult)
    for ht in range(2):
        nc.vector.tensor_scalar_mul(
            out=vq2e[:, ht], in0=scr[:, ht], scalar1=e.ve21[:, ht : ht + 1])
        nc.vector.tensor_scalar_mul(
            out=vqe[:, ht], in0=qk_sb[:, 0, ht],
            scalar1=e.ve12[:, ht : ht + 1])
    nc.vector.tensor_tensor(
        out=k2[:], in0=qk_sb[:, 1], in1=qk_sb[:, 1], op=OP.mult)
    nc.vector.tensor_scalar(
        out=scr[:], in0=k2[:], scalar1=PC3, scalar2=PC1,
        op0=OP.mult, op1=OP.add)
    nc.vector.tensor_tensor(
        out=scr[:], in0=qk_sb[:, 1], in1=scr[:], op=OP.mult)
    for ht in range(2):
        nc.vector.tensor_scalar_mul(
            out=vcombo[:, ht], in0=scr[:, ht], scalar1=e.vch[:, ht : ht + 1])
    nc.vector.tensor_tensor(
        out=ck2[:], in0=ck_sb[:], in1=ck_sb[:], op=OP.mult)
    nc.vector.tensor_scalar(
        out=scr2[:], in0=ck2[:], scalar1=PC3, scalar2=PC1,
        op0=OP.mult, op1=OP.add)
    nc.vector.tensor_tensor(
        out=scr2[:], in0=ck_sb[:], in1=scr2[:], op=OP.mult)
    for ht in range(2):
        nc.vector.tensor_scalar_mul(
            out=cvcombo[:, ht], in0=scr2[:, ht],
            scalar1=e.cavch[:, ht : ht + 1])
    return qk_sb, ck_sb, (vq2e, vqe, k2, vcombo, ck2, cvcombo)


def _attn_stage(e, sg, tmp2, qk_sb, ck_sb, pows):
    nc = e.nc
    vq2e, vqe, k2, vcombo, ck2, cvcombo = pows
    ar = e.arp.tile([128, 4, 2, 34, 8], FP8, tag="ar")
    nc.vector.memset(ar[:, :, :, 0:1, :], 0.0)
    nc.vector.memset(ar[:, :, :, 33:34, :], 0.0)
    for m in range(2):
        nc.vector.tensor_copy(
            out=ar[:, 0, m, 1:33, :],
            in_=tmp2[:, m, :, :].rearrange("p s u -> p u s"))

    for g in range(2):
        g4 = slice(g * 4, g * 4 + 4)
        # tmp2 transposed for 4 segments at once: [(4s,k), m, c]
        t2kc4 = e.atp.tile([128, 2, 128], BF16, tag="t2kc4")
        for m in range(2):
            tp = e.tps.tile([128, 128], BF16, tag="tp")
            nc.tensor.transpose(tp[:], tmp2[:, m, g4, :], e.ident[:])
            nc.vector.tensor_copy(out=t2kc4[:, m, :], in_=tp[:])

        # scores for 4 segments in one tile [(4s,q), (4s,k)]
        s_ps = e.sps.tile([128, 128], F32, tag="s")
        n = 0
        for ht in range(2):
            for lhsT, rhs in (
                (vq2e[:, ht, g4, :], qk_sb[:, 1, ht, g4, :]),
                (vqe[:, ht, g4, :], k2[:, ht, g4, :]),
                (e.ones[:], vcombo[:, ht, g4, :]),
            ):
                nc.tensor.matmul(s_ps[:], lhsT, rhs,
                                 start=(n == 0), stop=(n == 5))
                n += 1
        sc_ps = e.sps.tile([2, 128], F32, tag="sc")
        n = 0
        for ht in range(2):
            for lhsT, rhs in (
                (e.cA[:, ht, 0, :], ck_sb[:, ht, g4, :]),
                (e.cA[:, ht, 1, :], ck2[:, ht, g4, :]),
                (e.cA[:, ht, 2, :], cvcombo[:, ht, g4, :]),
            ):
                nc.tensor.matmul(sc_ps[:], lhsT, rhs,
                                 start=(n == 0), stop=(n == 5))
                n += 1

        # masked softmax; off-diagonal seg blocks are killed by the mask
        nc.vector.tensor_tensor(
            out=s_ps[:], in0=s_ps[:], in1=e.mb4[:, sg, g, :], op=OP.add)
        nc.scalar.activation(out=s_ps[:], in_=s_ps[:], func=ACTF.Exp)
        zs = e.smp.tile([128, 1], F32, tag="zs")
        nc.vector.reduce_sum(out=zs[:], in_=s_ps[:], axis=AX.X)
        nc.vector.reciprocal(out=zs[:], in_=zs[:])
        a_sb = e.smp.tile([128, 128], BF16, tag="a_sb")
        nc.vector.tensor_scalar_mul(
            out=a_sb[:], in0=s_ps[:], scalar1=zs[:, 0:1])

        nc.vector.tensor_tensor(
            out=sc_ps[:], in0=sc_ps[:], in1=e.mc4[:, sg, g, :], op=OP.add)
        nc.scalar.activation(out=sc_ps[:], in_=sc_ps[:], func=ACTF.Exp)
        scv = sc_ps[:].rearrange("p (s k) -> p s k", k=L4)
        zc = e.smp.tile([2, 4], F32, tag="zc")
        nc.vector.reduce_sum(out=zc[:], in_=scv[:], axis=AX.X)
        nc.vector.reciprocal(out=zc[:], in_=zc[:])
        ac_sb = e.smp.tile([2, 4, L4], BF16, tag="ac_sb")
        nc.vector.tensor_tensor(
            out=ac_sb[:], in0=scv[:],
            in1=zc[:].unsqueeze(2).broadcast_to([2, 4, L4]), op=OP.mult)

        # one transpose each -> block-diagonal rhs [(4s,k), (4s,q)+(4s,ci)]
        rhs136 = e.atp.tile([128, 136], BF16, tag="rhs136")
        aT_ps = e.tps.tile([128, 128], BF16, tag="tp")
        nc.tensor.transpose(aT_ps[:], a_sb[:], e.ident[:])
        nc.vector.tensor_copy(out=rhs136[:, 0:128], in_=aT_ps[:])
        acT_ps = e.tps.tile([128, 2], BF16, tag="tp")
        nc.tensor.transpose(
            acT_ps[:], ac_sb[:].rearrange("p s k -> p (s k)"),
            e.ident[0:2, 0:2])
        nc.vector.tensor_tensor(
            out=rhs136[:, 128:136].rearrange("p (s c) -> p s c", c=2),
            in0=acT_ps[:].unsqueeze(1).broadcast_to([128, 4, 2]),
            in1=e.blk[:], op=OP.mult)

        # batched self+concept attention results: [c, (4s,q)] + [c, (4s,ci)]
        srp = e.wps.tile([128, 2, 136], F32, tag="work")
        for m in range(2):
            nc.tensor.matmul(srp[:, m, :], t2kc4[:, m, :], rhs136[:],
                             start=(m == 0), stop=(m == 1))
        for m in range(2):
            nc.vector.tensor_copy(
                out=ar[:, 1, m, 1:33, g4],
                in_=srp[:, m, 0:128].rearrange("p (s q) -> p q s", q=L4))
            nc.vector.tensor_copy(
                out=ar[:, 2:4, m, 1:33, g4],
                in_=srp[:, m, 128:136].rearrange("p (s c) -> p c s", c=2)
                    .unsqueeze(2).broadcast_to([128, 2, L4, 4]))
    return ar


def _deconv_score_stage(e, sg, ar):
    nc = e.nc
    # deconv1: [1024,32] -> [512,64] (seg-minor)
    r1t = e.r1p.tile([128, 2, 2, 66, 8], FP8, tag="r1t")
    nc.vector.memset(r1t[:, :, :, 0:1, :], 0.0)
    nc.vector.memset(r1t[:, :, :, 65:66, :], 0.0)
    for m in range(4):
        for par, taps in DC_TAPS:
            d1 = e.wps.tile([128, L4, 8], F32, tag="work")
            n_mm = 0
            for kcp in range(4):
                for t, off in taps:
                    nc.tensor.matmul(
                        d1[:], e.dc1w8[:, kcp, :, t, m * 128 : (m + 1) * 128],
                        ar[:, kcp, :, off : off + 32, :],
                        start=(n_mm == 0), stop=(n_mm == 7),
                        perf_mode=DR)
                    n_mm += 1
            nc.scalar.activation(
                out=r1t[:, m // 2, m % 2, 1 + par : 65 + par : 2, :],
                in_=d1[:], func=ACTF.Identity,
                bias=e.dc1b[:, m : m + 1], scale=ISC)

    # deconv2: [512,64] -> [128,128] (r2t: [p, pos128, seg8])
    r2t = e.r2p.tile([128, 128, 8], BF16, tag="r2t")
    for par, taps in DC_TAPS:
        d2 = e.wps.tile([128, 64, 8], F32, tag="work")
        n_mm = 0
        for kcp in range(2):
            for t, off in taps:
                for h in range(2):     # same weights back-to-back
                    nc.tensor.matmul(
                        d2[:, h * 32 : (h + 1) * 32, :],
                        e.dc2w8[:, kcp, :, t, :],
                        r1t[:, kcp, :, off + h * 32 : off + h * 32 + 32, :],
                        start=(n_mm == 0), stop=(n_mm == 7),
                        perf_mode=DR)
                    n_mm += 1
        nc.scalar.activation(
            out=r2t[:, par : 128 : 2, :], in_=d2[:], func=ACTF.Identity,
            bias=e.dc2b[:, 0:1], scale=ISC)

    # scoring
    score_ps = e.scp.tile([16, 128], F32, tag="score")   # [(sub,ci), l]
    for sub in range(8):
        sim_ps = e.wps.tile([128, 128], F32, tag="work")
        nc.tensor.matmul(sim_ps[:], e.s1w[:], r2t[:, :, sub],
                         start=True, stop=True)
        sim_sb = e.smp.tile([128, 128], BF16, tag="sim_sb")
        nc.vector.tensor_copy(out=sim_sb[:], in_=sim_ps[:])
        nc.tensor.matmul(
            score_ps[:],
            e.uband[:, (128 - sub) * 2 : (128 - sub) * 2 + 16], sim_sb[:],
            start=(sub == 0), stop=(sub == 7))
    # sigmoid(z+b) = 0.5 + 0.5*tanh((z+b)/2): stays in the tanh func set,
    # avoiding per-supergroup activation-table reloads (mlpb holds b/2)
    final = e.finp.tile([16, 128], F32, tag="final")
    nc.scalar.activation(out=final[:], in_=score_ps[:], func=ACTF.Tanh,
                         bias=e.mlpb[:, 0:1], scale=0.5)
    nc.vector.tensor_scalar(out=final[:], in0=final[:], scalar1=0.5,
                            scalar2=0.5, op0=OP.mult, op1=OP.add)
    nc.sync.dma_start(
        out=e.io["out"].transpose([1, 0, 2])[sg * 8 : sg * 8 + 8],
        in_=final[:])


def _late_weights(e):
    nc = e.nc
    for t_sb, name in [(e.dc1w8, "dc1w"), (e.dc2w8, "dc2w"), (e.s1w, "s1w")]:
        nc.sync.dma_start(out=t_sb[:], in_=e.io[name])


def _body(e, first=False):
    _phase0(e)
    tmp2_cur = _conv_stage(e, 0, xgs=e.xg0 if first else None)
    if first:
        _late_weights(e)
    for sg in range(4):
        qk_sb, ck_sb, pows = _proj_stage(e, tmp2_cur)
        tmp2_next = _conv_stage(e, sg + 1) if sg < 3 else None
        ar = _attn_stage(e, sg, tmp2_cur, qk_sb, ck_sb, pows)
        _deconv_score_stage(e, sg, ar)
        tmp2_cur = tmp2_next


def _emit(ctx, tc, io, reps=1, loop_reps=0):
    e = _setup(ctx, tc, io)
    if loop_reps:
        # timing-only variant: body wrapped in a hardware loop so device
        # time dominates RPC noise; late weights hoisted before the loop
        _late_weights(e)
        with tc.For_i(0, loop_reps):
            _body(e, first=False)
        return
    for _rep in range(reps):
        _body(e, first=(_rep == 0))


# ---------------------------------------------------------------------------
# program build (cached)
# ---------------------------------------------------------------------------

_CACHE = {}


def _build(reps=1, loop_reps=0):
    key = ("nc", reps, loop_reps)
    if key in _CACHE:
        return _CACHE[key]
    nc = bacc.Bacc("TRN2", target_bir_lowering=False, debug=False)
    d = {}

    def di(name, shape, dt):
        d[name] = nc.dram_tensor(name, shape, dt, kind="ExternalInput").ap()

    di("x", [8, 128, 4, 2, 132, 4], FP8)
    di("c1w", [128, 4, 2, 5, C1], FP8)
    di("c2w", [128, 2, 2, 5, C2], FP8)
    di("dc1w", [128, 4, 2, 4, D1], FP8)
    di("dc2w", [128, 2, 2, 4, D2], FP8)
    di("saq", [128, 2, C2], BF16)
    di("sak", [128, 2, C2], BF16)
    di("cak", [128, 2, C2], BF16)
    di("caq", [128, 3, C2], BF16)
    di("s1w", [128, SIM], BF16)
    di("s2w", [128, 3, SIM], BF16)
    di("c1b", [128, 4], F32)
    di("c2b", [128, 2], F32)
    di("dc1b", [128, 4], F32)
    di("dc2b", [128, 1], F32)
    di("sbqk", [128, 2], F32)
    di("cab", [128, 2], F32)
    di("mlpw", [128, 1], F32)
    di("mlpb", [16, 1], F32)
    di("cvec", [128, 3, 2], BF16)
    di("ve21", [128, 2], F32)
    di("ve12", [128, 2], F32)
    di("vch", [128, 2], F32)
    di("cve21", [128, 2], F32)
    di("cve12", [128, 2], F32)
    di("cavch", [128, 2], F32)
    di("blk", [128, 4, 2], BF16)
    di("mb4", [128, 4, 2, 128], F32)
    di("mc4", [2, 4, 2, 128], F32)
    d["out"] = nc.dram_tensor("out", [2, SEG, 128], F32,
                              kind="ExternalOutput").ap()
    with tile.TileContext(nc) as tc:
        with ExitStack() as ctx:
            _emit(ctx, tc, d, reps=reps, loop_reps=loop_reps)
    nc.compile()
    _CACHE[key] = nc
    return nc


# ---------------------------------------------------------------------------
# host-side prep (layout/cast only)
# ---------------------------------------------------------------------------

def _chunk_bias(v, nchunk):
    return np.ascontiguousarray(
        np.asarray(v, np.float32).reshape(nchunk, 128).T)


def _band(v, dtype=None, scale=1.0):
    # [128, 2, 256]; column 128 of chunk ht = v[ht*128:(ht+1)*128]
    dtype = dtype or nbf
    out = np.zeros((128, 2, 256), dtype)
    vv = np.asarray(v, np.float32).reshape(2, 128).T * scale
    out[:, :, 128] = vv.astype(dtype)
    return out


def _wchunks(w, nk, dtype=None, scale=1.0):
    # w: [K, ...] -> [128, nk, ...] (zero-pad K up to nk*128)
    w = np.asarray(w, np.float32) * scale
    k = w.shape[0]
    if k < nk * 128:
        w = np.concatenate(
            [w, np.zeros((nk * 128 - k,) + w.shape[1:], np.float32)], 0)
    w = w.reshape((nk, 128) + w.shape[1:])
    perm = (1, 0) + tuple(range(2, w.ndim))
    return np.ascontiguousarray(w.transpose(perm)).astype(dtype or nbf)


def prepare_common(inp):
    g = {}
    g["c1w"] = _wchunks(np.asarray(inp["conv1_w"], np.float32)
                        .transpose(1, 2, 0), 8, nf8, SC).reshape(
                            128, 4, 2, 5, C1)
    g["c2w"] = _wchunks(np.asarray(inp["conv2_w"], np.float32)
                        .transpose(1, 2, 0), 4, nf8, SC).reshape(
                            128, 2, 2, 5, C2)
    g["dc1w"] = _wchunks(np.asarray(inp["dc1_w"], np.float32)
                         .transpose(0, 2, 1), 8, nf8, SC).reshape(
                             128, 4, 2, 4, D1)
    g["dc2w"] = _wchunks(np.asarray(inp["dc2_w"], np.float32)
                         .transpose(0, 2, 1), 4, nf8, SC).reshape(
                             128, 2, 2, 4, D2)
    g["saq"] = _wchunks(inp["sa_wq"], 2)
    g["sak"] = _wchunks(inp["sa_wk"], 2)
    g["cak"] = _wchunks(inp["ca_wk"], 2)
    g["caq"] = _wchunks(inp["ca_wq"], 3)              # [128,3,256]
    g["s1w"] = np.ascontiguousarray(
        np.asarray(inp["sim1_w"], np.float32)).astype(nbf)
    g["s2w"] = _wchunks(inp["sim2_w"], 3)             # [128,3,128]
    vv = np.ascontiguousarray(
        np.asarray(inp["sa_v"], np.float32).reshape(2, 128).T)
    g["ve21"] = PE3 * vv
    g["ve12"] = PE3 * vv
    g["vch"] = vv
    cvv = np.ascontiguousarray(
        np.asarray(inp["ca_v"], np.float32).reshape(2, 128).T)
    g["cve21"] = PE3 * cvv
    g["cve12"] = PE3 * cvv
    g["cavch"] = cvv
    blk = np.zeros((128, 4, 2), np.float32)
    for s in range(4):
        blk[s * 32 : (s + 1) * 32, s, :] = 1.0
    g["blk"] = blk.astype(nbf)
    g["c1b"] = _chunk_bias(inp["conv1_b"], 4)
    g["c2b"] = _chunk_bias(inp["conv2_b"], 2)
    g["dc1b"] = _chunk_bias(inp["dc1_b"], 4)
    g["dc2b"] = _chunk_bias(inp["dc2_b"], 1)
    g["sbqk"] = _chunk_bias(
        np.asarray(inp["sa_bq"], np.float32)
        + np.asarray(inp["sa_bk"], np.float32), 2)
    g["cab"] = _chunk_bias(
        np.asarray(inp["ca_bq"], np.float32)
        + np.asarray(inp["ca_bk"], np.float32), 2)
    g["mlpw"] = np.ascontiguousarray(
        np.asarray(inp["mlp_w"], np.float32).reshape(128, 1))
    g["mlpb"] = np.full((16, 1), float(np.asarray(inp["mlp_b"])) * 0.5,
                        np.float32)
    return g


def prepare_core(inp, b):
    o = {}
    x = np.asarray(inp["batch"], np.float32)[b]       # [32,128,1024]
    x = x.transpose(0, 2, 1)                          # [32,1024,128]
    xp = np.zeros((SEG, CIN, 132), np.float32)
    xp[:, :, 2:130] = x
    xp = xp.reshape(8, 4, 8, 128, 132).transpose(0, 3, 2, 4, 1)
    o["x"] = np.ascontiguousarray(xp).astype(nf8).reshape(
        8, 128, 4, 2, 132, 4)    # [grp, p, kcp, two, pos, seg] seg-minor
    cv = np.zeros((2, 384), np.float32)
    cv[0, :CD] = np.asarray(inp["concept1"], np.float32)[b]
    cv[1, :CD] = np.asarray(inp["concept2"], np.float32)[b]
    o["cvec"] = np.ascontiguousarray(
        cv.reshape(2, 3, 128).transpose(2, 1, 0)).astype(nbf)  # [128,3,2]
    sl = np.asarray(inp["seg_len"], np.int64)[b]      # [32]
    k = np.arange(L4)
    mrow = np.where(sl[:, None] > 4 * k[None, :], 0.0, NEG).astype(np.float32)
    mb4 = np.full((128, 4, 2, 128), NEG, np.float32)
    mc4 = np.empty((2, 4, 2, 128), np.float32)
    for sg in range(4):
        for g in range(2):
            base = sg * 8 + g * 4
            for s in range(4):
                mb4[s * 32 : (s + 1) * 32, sg, g, s * 32 : (s + 1) * 32] = (
                    mrow[base + s][None, :])
            mc4[:, sg, g, :] = mrow[base : base + 4].reshape(1, 128)
    o["mb4"] = mb4
    o["mc4"] = mc4
    return o


def kernel(**inputs):
    nc = _build()
    common = prepare_common(inputs)
    in_maps = []
    for b in range(B):
        m = dict(common)
        m.update(prepare_core(inputs, b))
        in_maps.append(m)
    res = run_bass_kernel_spmd(nc, in_maps, list(range(B)))
    s1 = np.stack([res.results[b]["out"][0] for b in range(B)])
    s2 = np.stack([res.results[b]["out"][1] for b in range(B)])
    return s1.astype(np.float32), s2.astype(np.float32)



# revision 28
# speedup vs baseline: 1.2013x; 1.2013x over previous
"""Trainium2 Bass kernel for nn_CHAN_29764123361280 (ragged_sequence).

Sharding: data-parallel over the batch axis B=8 -> one video per NeuronCore,
all weights replicated. Per-core pipeline (32 segments):
  conv1(k5,p2)+maxpool2 -> conv2(k5,p2)+maxpool2 -> additive self-attention
  + two concept attentions -> concat -> deconv1 -> deconv2 -> similarity
  scoring.  All matmuls bf16/fp8 with fp32 PSUM accumulation.

The additive-attention tanh is replaced by an odd cubic polynomial
(tanh(z) ~ c1 z + c3 z^3, weighted-LSQ fit on |z|<=5.2).  Expanding
(qp+kp)^3 turns the score tensor into a sum of rank-256 bilinear forms:
  s[q,k] = <v o qp^2, kp>*3c3 + <v o qp, kp^2>*3c3 + <v, c1 kp + c3 kp^3>
(pure-q terms are softmax-invariant and dropped), i.e. six N=128 matmuls
per 4-segment group instead of the L4^2*H elementwise tanh monster.
End-to-end rel-err of this approximation is ~1.8e-4, far below the fp8
conv noise (~1.6e-3).

Scores for 4 segments land in one PSUM tile [(4s,q)=128, (4s,k)=128];
cross-segment blocks are killed by a block-diagonal mask before the
softmax, so one PE transpose of the softmaxed tile directly yields the
block-diagonal rhs for a batched self-attention-result matmul.

The conv stage for supergroup sg+1 is emitted between the projection and
attention phases of supergroup sg so the PE has conv work while the
attention chain runs.
"""

from contextlib import ExitStack

import numpy as np
import ml_dtypes

import concourse.bass as bass  # noqa: F401
import concourse.mybir as mybir
import concourse.tile as tile
from concourse import bacc
from concourse.bass_utils import run_bass_kernel_spmd
from concourse.masks import make_identity

BF16 = mybir.dt.bfloat16
F32 = mybir.dt.float32
FP8 = mybir.dt.float8e4
DR = mybir.MatmulPerfMode.DoubleRow
SC = 256.0           # fp8 weight pre-scale (undone via activation scale)
ISC = 1.0 / SC
nf8 = ml_dtypes.float8_e4m3

B, S, L, CIN = 8, 32, 128, 1024
C1, C2 = 512, 256
D1, D2 = 512, 128
CD, SIM = 300, 128
L4 = L // 4          # 32
SEG = S              # segments per core
NEG = -30.0          # mask logit bias (exp(-30) ~ 1e-13)

nbf = ml_dtypes.bfloat16

DC_TAPS = ((0, ((1, 1), (3, 0))), (1, ((2, 1), (0, 2))))
# parity -> ((tap, input col offset), ...) for ConvTranspose1d(k=4,s=2,p=1)
# on halo'd input xh[:, u+1] = x[:, u]:
#   even out j=2u:  W1.x[u]  + W3.x[u-1]
#   odd  out j=2u+1: W2.x[u] + W0.x[u+1]

# odd-cubic tanh fit (weighted LSQ on |z|<=5.2; z=qp+kp stays within ~4.7)
PC1 = 0.63982097
PC3 = -0.02262075
PE3 = 3.0 * PC3          # coefficient of the q^2 k / q k^2 cross terms

AX = mybir.AxisListType
OP = mybir.AluOpType
ACTF = mybir.ActivationFunctionType


class _Env:
    pass


def _setup(ctx, tc, io):
    nc = tc.nc
    e = _Env()
    e.nc, e.tc, e.io = nc, tc, io
    singles = ctx.enter_context(tc.tile_pool(name="singles", bufs=1))
    e.singles = singles
    # ---- resident weights / constants ----
    # conv/deconv weights fp8 (x256), paired on dim1+2 for DoubleRow
    e.c1w8 = singles.tile([128, 4, 2, 5, C1], FP8)
    e.c2w8 = singles.tile([128, 2, 2, 5, C2], FP8)
    e.dc1w8 = singles.tile([128, 4, 2, 4, D1], FP8)
    e.dc2w8 = singles.tile([128, 2, 2, 4, D2], FP8)
    e.saq = singles.tile([128, 2, C2], BF16)
    e.sak = singles.tile([128, 2, C2], BF16)
    e.cak = singles.tile([128, 2, C2], BF16)
    e.caq = singles.tile([128, 3, C2], BF16)
    e.s1w = singles.tile([128, SIM], BF16)
    e.s2w = singles.tile([128, 3, SIM], BF16)
    e.c1b = singles.tile([128, 4], F32)
    e.c2b = singles.tile([128, 2], F32)
    e.dc1b = singles.tile([128, 4], F32)
    e.dc2b = singles.tile([128, 1], F32)
    e.sbqk = singles.tile([128, 2], F32)
    e.cab = singles.tile([128, 2], F32)
    e.mlpw = singles.tile([128, 1], F32)
    e.mlpb = singles.tile([16, 1], F32)
    e.cvec = singles.tile([128, 3, 2], BF16)
    e.ve21 = singles.tile([128, 2], F32)    # 3*c3 * sa_v   (h chunks)
    e.ve12 = singles.tile([128, 2], F32)
    e.vch = singles.tile([128, 2], F32)     # sa_v
    e.cve21 = singles.tile([128, 2], F32)   # 3*c3 * ca_v
    e.cve12 = singles.tile([128, 2], F32)
    e.cavch = singles.tile([128, 2], F32)   # ca_v
    e.mb4 = singles.tile([128, 4, 2, 128], F32)  # [(4s,q), sg, g, (4s,k)]
    e.mc4 = singles.tile([2, 4, 2, 128], F32)    # [ci, sg, g, (4s,k)]
    e.blk = singles.tile([128, 4, 2], BF16)  # 0/1 seg-block pattern
    e.ones = singles.tile([128, 128], BF16)
    e.ident = singles.tile([128, 128], BF16)

    # ---- pools ----
    e.xp = ctx.enter_context(tc.tile_pool(name="xp", bufs=3))
    e.t1p = ctx.enter_context(tc.tile_pool(name="t1p", bufs=3))
    e.t2p = ctx.enter_context(tc.tile_pool(name="t2p", bufs=2))
    e.atp = ctx.enter_context(tc.tile_pool(name="atp", bufs=2))
    e.powp = ctx.enter_context(tc.tile_pool(name="powp", bufs=2))
    e.arp = ctx.enter_context(tc.tile_pool(name="arp", bufs=2))
    e.r1p = ctx.enter_context(tc.tile_pool(name="r1p", bufs=2))
    e.r2p = ctx.enter_context(tc.tile_pool(name="r2p", bufs=2))
    e.smp = ctx.enter_context(tc.tile_pool(name="smp", bufs=3))
    e.php = ctx.enter_context(tc.tile_pool(name="php", bufs=1))
    e.finp = ctx.enter_context(tc.tile_pool(name="finp", bufs=2))

    # PSUM: conv/work(2x1) + s(1) + sc(1) + transp(2x1) + score/sim(1)
    e.wps = ctx.enter_context(tc.tile_pool(name="wps", bufs=2, space="PSUM"))
    e.sps = ctx.enter_context(tc.tile_pool(name="sps", bufs=1, space="PSUM"))
    e.tps = ctx.enter_context(tc.tile_pool(name="tps", bufs=2, space="PSUM"))
    e.scp = ctx.enter_context(tc.tile_pool(name="scp", bufs=1, space="PSUM"))

    e.xg0 = []
    for g in range(2):
        xg = e.xp.tile([128, 4, 2, 132, 4], FP8, name=f"xg0{g}", tag="xg")
        e.xg0.append(xg)
    # startup-critical DMAs on separate engine queues so the transfers
    # overlap instead of serializing on one queue
    nc.sync.dma_start(out=e.xg0[0][:], in_=io["x"][0])
    nc.scalar.dma_start(out=e.c1w8[:, 0], in_=io["c1w"][:, 0])
    nc.gpsimd.dma_start(out=e.c1w8[:, 1], in_=io["c1w"][:, 1])
    nc.scalar.dma_start(out=e.c1w8[:, 2], in_=io["c1w"][:, 2])
    nc.gpsimd.dma_start(out=e.c1w8[:, 3], in_=io["c1w"][:, 3])
    nc.sync.dma_start(out=e.xg0[1][:], in_=io["x"][1])
    for t_sb, name in [
        (e.cvec, "cvec"), (e.caq, "caq"), (e.s2w, "s2w"),
        (e.c1b, "c1b"), (e.c2b, "c2b"), (e.dc1b, "dc1b"), (e.dc2b, "dc2b"),
        (e.sbqk, "sbqk"), (e.cab, "cab"), (e.mlpw, "mlpw"),
        (e.mlpb, "mlpb"), (e.mb4, "mb4"), (e.mc4, "mc4"),
        (e.ve21, "ve21"), (e.ve12, "ve12"), (e.vch, "vch"),
        (e.cve21, "cve21"), (e.cve12, "cve12"), (e.cavch, "cavch"),
        (e.blk, "blk"),
    ]:
        nc.sync.dma_start(out=t_sb[:], in_=io[name])
    nc.sync.dma_start(out=e.c2w8[:], in_=io["c2w"])
    nc.sync.dma_start(out=e.saq[:], in_=io["saq"])
    nc.sync.dma_start(out=e.sak[:], in_=io["sak"])
    nc.sync.dma_start(out=e.cak[:], in_=io["cak"])
    make_identity(nc, e.ident[:])
    nc.vector.memset(e.ones[:], 1.0)

    # Touch every DMA'd tensor an engine will read, one instruction per
    # tensor, so each engine's vector clock observes the DMA semaphores
    # early: later compute then never needs >1 sync wait per instruction
    # (the walrus TT/STT encodings only carry one).
    e.dve_scr = singles.tile([1, 16], F32)
    e.act_scr = singles.tile([1, 16], F32)
    for i, t_sb in enumerate(
            (e.cab, e.mlpw, e.mb4[:, 0, 0, :], e.dc1b, e.dc2b, e.sbqk,
             e.c1b, e.c2b, e.mlpb, e.mc4[:, 0, 0, :], e.ve21, e.ve12,
             e.vch, e.blk[:, 0, :])):
        nc.vector.tensor_copy(out=e.dve_scr[0:1, i : i + 1],
                              in_=t_sb[0:1, 0:1])
        nc.scalar.copy(out=e.act_scr[0:1, i : i + 1], in_=t_sb[0:1, 0:1])

    return e


def _phase0(e):
    nc = e.nc
    e.cA = e.php.tile([128, 2, 3, 2], BF16)    # concept lhsT stacks
    e.uband = e.php.tile([128, 512], BF16)     # banded (ci-interleaved) u
    cqb = e.php.tile([128, 2, 2], F32)
    cq2 = e.php.tile([128, 2, 2], F32)
    cq_ps = e.wps.tile([128, 2, 2], F32, tag="work")
    for i, (ci, ht) in enumerate([(c, h) for c in range(2) for h in range(2)]):
        for kc in range(3):
            nc.tensor.matmul(
                cq_ps[:, ht, ci : ci + 1],
                e.caq[:, kc, ht * 128 : (ht + 1) * 128],
                e.cvec[:, kc, ci : ci + 1],
                start=(i == 0 and kc == 0), stop=(i == 3 and kc == 2))
    for ci in range(2):
        for ht in range(2):
            nc.vector.tensor_tensor(
                out=cqb[:, ht, ci : ci + 1], in0=cq_ps[:, ht, ci : ci + 1],
                in1=e.cab[:, ht : ht + 1], op=OP.add)
    nc.vector.tensor_tensor(out=cq2[:], in0=cqb[:], in1=cqb[:], op=OP.mult)
    for ht in range(2):
        nc.vector.tensor_scalar_mul(
            out=e.cA[:, ht, 0, :], in0=cq2[:, ht, :],
            scalar1=e.cve21[:, ht : ht + 1])
        nc.vector.tensor_scalar_mul(
            out=e.cA[:, ht, 1, :], in0=cqb[:, ht, :],
            scalar1=e.cve12[:, ht : ht + 1])
        nc.vector.tensor_copy(
            out=e.cA[:, ht, 2, :],
            in_=e.cavch[:, ht : ht + 1].broadcast_to([128, 2]))
    cw_ps = e.wps.tile([128, 2], F32, tag="work")
    for ci in range(2):
        for kc in range(3):
            nc.tensor.matmul(
                cw_ps[:, ci : ci + 1], e.s2w[:, kc, :],
                e.cvec[:, kc, ci : ci + 1],
                start=(ci == 0 and kc == 0), stop=(ci == 1 and kc == 2))
    nc.vector.memset(e.uband[:], 0.0)
    for ci in range(2):
        nc.vector.tensor_tensor(
            out=e.uband[:, 256 + ci : 257 + ci], in0=cw_ps[:, ci : ci + 1],
            in1=e.mlpw[:, 0:1], op=OP.mult)
    nc.scalar.copy(out=e.act_scr[0:1, 15:16], in_=e.cA[0:1, 0, 0, 0:1])
    e.tc.no_sync_barrier()


def _conv_stage(e, sg, xgs=None):
    """conv1+pool+conv2+pool for supergroup sg -> tmp2 [128, 2, 8, 32]."""
    nc = e.nc
    tmp2 = e.t2p.tile([128, 2, 8, L4], BF16, tag="tmp2")
    for g in range(2):
        if xgs is not None:
            xg = xgs[g]
        else:
            xg = e.xp.tile([128, 4, 2, 132, 4], FP8, tag="xg")
            nc.sync.dma_start(out=xg[:], in_=e.io["x"][sg * 2 + g])

        # seg-minor layouts: free cols = (pos, seg) so the DoubleRow
        # moving AP stays 3-dim [K, 2, N]
        t1 = e.t1p.tile([128, 2, 2, 68, 4], FP8, tag="t1")
        nc.vector.memset(t1[:, :, :, 0:2, :], 0.0)
        nc.vector.memset(t1[:, :, :, 66:68, :], 0.0)
        for m in range(4):
            y1 = e.wps.tile([128, 128, 4], F32, tag="conv")
            n_mm = 0
            for kcp in range(4):
                for t in range(5):
                    for h in range(2):     # same weights back-to-back
                        nc.tensor.matmul(
                            y1[:, h * 64 : (h + 1) * 64, :],
                            e.c1w8[:, kcp, :, t, m * 128 : (m + 1) * 128],
                            xg[:, kcp, :, t + h * 64 : t + h * 64 + 64, :],
                            start=(n_mm == 0), stop=(n_mm == 39),
                            perf_mode=DR)
                        n_mm += 1
            ys = e.t1p.tile([128, 128, 4], F32, tag="pool1")
            nc.scalar.activation(out=ys[:], in_=y1[:], func=ACTF.Identity,
                                 bias=e.c1b[:, m : m + 1], scale=ISC)
            yv = ys[:].rearrange("p (u two) s -> p u two s", two=2)
            nc.vector.tensor_tensor(
                out=t1[:, m // 2, m % 2, 2:66, :], in0=yv[:, :, 0, :],
                in1=yv[:, :, 1, :], op=OP.max)

        for m in range(2):
            y2 = e.wps.tile([128, 64, 4], F32, tag="conv")
            n_mm = 0
            for kcp in range(2):
                for t in range(5):
                    nc.tensor.matmul(
                        y2[:], e.c2w8[:, kcp, :, t, m * 128 : (m + 1) * 128],
                        t1[:, kcp, :, t : t + 64, :],
                        start=(n_mm == 0), stop=(n_mm == 9),
                        perf_mode=DR)
                    n_mm += 1
            ys = e.t1p.tile([128, 64, 4], F32, tag="pool2")
            nc.scalar.activation(out=ys[:], in_=y2[:], func=ACTF.Identity,
                                 bias=e.c2b[:, m : m + 1], scale=ISC)
            yv = ys[:].rearrange("p (u two) s -> p u two s", two=2)
            nc.vector.tensor_tensor(
                out=tmp2[:, m, g * 4 : g * 4 + 4, :],
                in0=yv[:, :, 0, :].rearrange("p u s -> p s u"),
                in1=yv[:, :, 1, :].rearrange("p u s -> p s u"), op=OP.max)
    return tmp2


def _proj_stage(e, tmp2):
    """qp/kp/ck projections (bf16 SBUF) + polynomial power tiles."""
    nc = e.nc
    qk_sb = e.atp.tile([128, 2, 2, 8, L4], BF16, tag="qk")  # [q/k, ht, s, q]
    ck_sb = e.atp.tile([128, 2, 8, L4], BF16, tag="ck")     # [ht, s, k]
    for ht in range(2):
        qp_ps = e.wps.tile([128, 8, L4], F32, tag="conv")
        for kc in range(2):
            nc.tensor.matmul(
                qp_ps[:], e.saq[:, kc, ht * 128 : (ht + 1) * 128],
                tmp2[:, kc, :, :], start=(kc == 0), stop=(kc == 1))
        nc.vector.tensor_scalar_add(
            out=qk_sb[:, 0, ht, :, :], in0=qp_ps[:],
            scalar1=e.sbqk[:, ht : ht + 1])
        kp_ps = e.wps.tile([128, 8, L4], F32, tag="conv")
        for kc in range(2):
            nc.tensor.matmul(
                kp_ps[:], e.sak[:, kc, ht * 128 : (ht + 1) * 128],
                tmp2[:, kc, :, :], start=(kc == 0), stop=(kc == 1))
        nc.vector.tensor_copy(out=qk_sb[:, 1, ht, :, :], in_=kp_ps[:])
        ck = e.wps.tile([128, 8, L4], F32, tag="work")
        for kc in range(2):
            nc.tensor.matmul(
                ck[:], e.cak[:, kc, ht * 128 : (ht + 1) * 128],
                tmp2[:, kc, :, :], start=(kc == 0), stop=(kc == 1))
        nc.vector.tensor_copy(out=ck_sb[:, ht, :, :], in_=ck[:])

    # polynomial stacks (bf16):
    #   lhsT chunks: vq2e = 3c3*(v o qp^2); vqe = 3c3*(v o qp); ones
    #   rhs  chunks: kp (raw); k2 = kp^2; vcombo = v o (c1 kp + c3 kp^3)
    pw = e.powp
    vq2e = pw.tile([128, 2, 8, L4], BF16, tag="vq2e")
    vqe = pw.tile([128, 2, 8, L4], BF16, tag="vqe")
    k2 = pw.tile([128, 2, 8, L4], BF16, tag="k2")
    vcombo = pw.tile([128, 2, 8, L4], BF16, tag="vcombo")
    ck2 = pw.tile([128, 2, 8, L4], BF16, tag="ck2")
    cvcombo = pw.tile([128, 2, 8, L4], BF16, tag="cvcombo")
    scr = pw.tile([128, 2, 8, L4], BF16, tag="pscr")
    scr2 = pw.tile([128, 2, 8, L4], BF16, tag="pscr2")
    nc.vector.tensor_tensor(
        out=scr[:], in0=qk_sb[:, 0], in1=qk_sb[:, 0], op=OP.mult)
    for ht in range(2):
        nc.vector.tensor_scalar_mul(
            out=vq2e[:, ht], in0=scr[:, ht], scalar1=e.ve21[:, ht : ht + 1])
        nc.vector.tensor_scalar_mul(
            out=vqe[:, ht], in0=qk_sb[:, 0, ht],
            scalar1=e.ve12[:, ht : ht + 1])
    nc.vector.tensor_tensor(
        out=k2[:], in0=qk_sb[:, 1], in1=qk_sb[:, 1], op=OP.mult)
    nc.vector.tensor_scalar(
        out=scr[:], in0=k2[:], scalar1=PC3, scalar2=PC1,
        op0=OP.mult, op1=OP.add)
    nc.vector.tensor_tensor(
        out=scr[:], in0=qk_sb[:, 1], in1=scr[:], op=OP.mult)
    for ht in range(2):
        nc.vector.tensor_scalar_mul(
            out=vcombo[:, ht], in0=scr[:, ht], scalar1=e.vch[:, ht : ht + 1])
    nc.vector.tensor_tensor(
        out=ck2[:], in0=ck_sb[:], in1=ck_sb[:], op=OP.mult)
    nc.vector.tensor_scalar(
        out=scr2[:], in0=ck2[:], scalar1=PC3, scalar2=PC1,
        op0=OP.mult, op1=OP.add)
    nc.vector.tensor_tensor(
        out=scr2[:], in0=ck_sb[:], in1=scr2[:], op=OP.mult)
    for ht in range(2):
        nc.vector.tensor_scalar_mul(
            out=cvcombo[:, ht], in0=scr2[:, ht],
            scalar1=e.cavch[:, ht : ht + 1])
    return qk_sb, ck_sb, (vq2e, vqe, k2, vcombo, ck2, cvcombo)


def _attn_stage(e, sg, tmp2, qk_sb, ck_sb, pows):
    nc = e.nc
    vq2e, vqe, k2, vcombo, ck2, cvcombo = pows
    ar = e.arp.tile([128, 4, 2, 34, 8], FP8, tag="ar")
    nc.vector.memset(ar[:, :, :, 0:1, :], 0.0)
    nc.vector.memset(ar[:, :, :, 33:34, :], 0.0)
    for m in range(2):
        nc.vector.tensor_copy(
            out=ar[:, 0, m, 1:33, :],
            in_=tmp2[:, m, :, :].rearrange("p s u -> p u s"))

    # tmp2 transposed for 4 segments at once: [(4s,k), m, c] -- hoisted
    # ahead of the softmax chains so the PE streams all 4 transposes while
    # group 0's softmax runs.
    t2kc4s = []
    for g in range(2):
        g4 = slice(g * 4, g * 4 + 4)
        t2kc4 = e.atp.tile([128, 2, 128], BF16, tag="t2kc4")
        for m in range(2):
            tp = e.tps.tile([128, 128], BF16, tag="tp")
            nc.tensor.transpose(tp[:], tmp2[:, m, g4, :], e.ident[:])
            nc.vector.tensor_copy(out=t2kc4[:, m, :], in_=tp[:])
        t2kc4s.append(t2kc4)

    for g in range(2):
        g4 = slice(g * 4, g * 4 + 4)
        t2kc4 = t2kc4s[g]
        # scores for 4 segments in one tile [(4s,q), (4s,k)]; concept
        # scores ride in columns 128:256 of the same PSUM bank (the PE
        # runs the groups in order, so the second group's has_written
        # clear cannot corrupt the first group's finished accumulation)
        s_ps = e.sps.tile([128, 256], F32, tag="s")
        n = 0
        for ht in range(2):
            for lhsT, rhs in (
                (vq2e[:, ht, g4, :], qk_sb[:, 1, ht, g4, :]),
                (vqe[:, ht, g4, :], k2[:, ht, g4, :]),
                (e.ones[:], vcombo[:, ht, g4, :]),
            ):
                nc.tensor.matmul(s_ps[:, 0:128], lhsT, rhs,
                                 start=(n == 0), stop=(n == 5))
                n += 1
        sc_ps = s_ps[0:2, 128:256]
        n = 0
        for ht in range(2):
            for lhsT, rhs in (
                (e.cA[:, ht, 0, :], ck_sb[:, ht, g4, :]),
                (e.cA[:, ht, 1, :], ck2[:, ht, g4, :]),
                (e.cA[:, ht, 2, :], cvcombo[:, ht, g4, :]),
            ):
                nc.tensor.matmul(sc_ps, lhsT, rhs,
                                 start=(n == 0), stop=(n == 5))
                n += 1

        # masked softmax; off-diagonal seg blocks are killed by the mask
        nc.vector.tensor_tensor(
            out=s_ps[:, 0:128], in0=s_ps[:, 0:128], in1=e.mb4[:, sg, g, :],
            op=OP.add)
        nc.scalar.activation(out=s_ps[:, 0:128], in_=s_ps[:, 0:128],
                             func=ACTF.Exp)
        zs = e.smp.tile([128, 1], F32, tag="zs")
        nc.vector.reduce_sum(out=zs[:], in_=s_ps[:, 0:128], axis=AX.X)
        nc.vector.reciprocal(out=zs[:], in_=zs[:])
        a_sb = e.smp.tile([128, 128], BF16, tag="a_sb")
        nc.vector.tensor_scalar_mul(
            out=a_sb[:], in0=s_ps[:, 0:128], scalar1=zs[:, 0:1])

        nc.vector.tensor_tensor(
            out=sc_ps, in0=sc_ps, in1=e.mc4[:, sg, g, :], op=OP.add)
        nc.scalar.activation(out=sc_ps, in_=sc_ps, func=ACTF.Exp)
        scv = sc_ps.rearrange("p (s k) -> p s k", k=L4)
        zc = e.smp.tile([2, 4], F32, tag="zc")
        nc.vector.reduce_sum(out=zc[:], in_=scv, axis=AX.X)
        nc.vector.reciprocal(out=zc[:], in_=zc[:])
        ac_sb = e.smp.tile([2, 4, L4], BF16, tag="ac_sb")
        nc.vector.tensor_tensor(
            out=ac_sb[:], in0=scv,
            in1=zc[:].unsqueeze(2).broadcast_to([2, 4, L4]), op=OP.mult)

        # one transpose each -> block-diagonal rhs [(4s,k), (4s,q)+(4s,ci)]
        rhs136 = e.atp.tile([128, 136], BF16, tag="rhs136")
        aT_ps = e.tps.tile([128, 128], BF16, tag="tp")
        nc.tensor.transpose(aT_ps[:], a_sb[:], e.ident[:])
        nc.vector.tensor_copy(out=rhs136[:, 0:128], in_=aT_ps[:])
        acT_ps = e.tps.tile([128, 2], BF16, tag="tp")
        nc.tensor.transpose(
            acT_ps[:], ac_sb[:].rearrange("p s k -> p (s k)"),
            e.ident[0:2, 0:2])
        nc.vector.tensor_tensor(
            out=rhs136[:, 128:136].rearrange("p (s c) -> p s c", c=2),
            in0=acT_ps[:].unsqueeze(1).broadcast_to([128, 4, 2]),
            in1=e.blk[:], op=OP.mult)

        # batched self+concept attention results: [c, (4s,q)] + [c, (4s,ci)]
        srp = e.wps.tile([128, 2, 136], F32, tag="work")
        for m in range(2):
            nc.tensor.matmul(srp[:, m, :], t2kc4[:, m, :], rhs136[:],
                             start=(m == 0), stop=(m == 1))
        for m in range(2):
            nc.vector.tensor_copy(
                out=ar[:, 1, m, 1:33, g4],
                in_=srp[:, m, 0:128].rearrange("p (s q) -> p q s", q=L4))
            nc.vector.tensor_copy(
                out=ar[:, 2:4, m, 1:33, g4],
                in_=srp[:, m, 128:136].rearrange("p (s c) -> p c s", c=2)
                    .unsqueeze(2).broadcast_to([128, 2, L4, 4]))
    return ar


def _deconv_score_stage(e, sg, ar):
    nc = e.nc
    # deconv1: [1024,32] -> [512,64] (seg-minor)
    r1t = e.r1p.tile([128, 2, 2, 66, 8], FP8, tag="r1t")
    nc.vector.memset(r1t[:, :, :, 0:1, :], 0.0)
    nc.vector.memset(r1t[:, :, :, 65:66, :], 0.0)
    for m in range(4):
        for par, taps in DC_TAPS:
            d1 = e.wps.tile([128, L4, 8], F32, tag="work")
            n_mm = 0
            for kcp in range(4):
                for t, off in taps:
                    nc.tensor.matmul(
                        d1[:], e.dc1w8[:, kcp, :, t, m * 128 : (m + 1) * 128],
                        ar[:, kcp, :, off : off + 32, :],
                        start=(n_mm == 0), stop=(n_mm == 7),
                        perf_mode=DR)
                    n_mm += 1
            nc.scalar.activation(
                out=r1t[:, m // 2, m % 2, 1 + par : 65 + par : 2, :],
                in_=d1[:], func=ACTF.Identity,
                bias=e.dc1b[:, m : m + 1], scale=ISC)

    # deconv2: [512,64] -> [128,128] (r2t: [p, pos128, seg8])
    r2t = e.r2p.tile([128, 128, 8], BF16, tag="r2t")
    for par, taps in DC_TAPS:
        d2 = e.wps.tile([128, 64, 8], F32, tag="work")
        n_mm = 0
        for kcp in range(2):
            for t, off in taps:
                for h in range(2):     # same weights back-to-back
                    nc.tensor.matmul(
                        d2[:, h * 32 : (h + 1) * 32, :],
                        e.dc2w8[:, kcp, :, t, :],
                        r1t[:, kcp, :, off + h * 32 : off + h * 32 + 32, :],
                        start=(n_mm == 0), stop=(n_mm == 7),
                        perf_mode=DR)
                    n_mm += 1
        nc.scalar.activation(
            out=r2t[:, par : 128 : 2, :], in_=d2[:], func=ACTF.Identity,
            bias=e.dc2b[:, 0:1], scale=ISC)

    # scoring
    score_ps = e.scp.tile([16, 128], F32, tag="score")   # [(sub,ci), l]
    for sub in range(8):
        sim_ps = e.wps.tile([128, 128], F32, tag="work")
        nc.tensor.matmul(sim_ps[:], e.s1w[:], r2t[:, :, sub],
                         start=True, stop=True)
        sim_sb = e.smp.tile([128, 128], BF16, tag="sim_sb")
        nc.vector.tensor_copy(out=sim_sb[:], in_=sim_ps[:])
        nc.tensor.matmul(
            score_ps[:],
            e.uband[:, (128 - sub) * 2 : (128 - sub) * 2 + 16], sim_sb[:],
            start=(sub == 0), stop=(sub == 7))
    # sigmoid(z+b) = 0.5 + 0.5*tanh((z+b)/2): stays in the tanh func set,
    # avoiding per-supergroup activation-table reloads (mlpb holds b/2)
    final = e.finp.tile([16, 128], F32, tag="final")
    nc.scalar.activation(out=final[:], in_=score_ps[:], func=ACTF.Tanh,
                         bias=e.mlpb[:, 0:1], scale=0.5)
    nc.vector.tensor_scalar(out=final[:], in0=final[:], scalar1=0.5,
                            scalar2=0.5, op0=OP.mult, op1=OP.add)
    nc.sync.dma_start(
        out=e.io["out"].transpose([1, 0, 2])[sg * 8 : sg * 8 + 8],
        in_=final[:])


def _late_weights(e):
    nc = e.nc
    for t_sb, name in [(e.dc1w8, "dc1w"), (e.dc2w8, "dc2w"), (e.s1w, "s1w")]:
        nc.sync.dma_start(out=t_sb[:], in_=e.io[name])


def _body(e, first=False):
    nc = e.nc
    if first:
        xgs0 = e.xg0
    else:
        # prefetch the first conv group's input before phase0 so the
        # iteration-start conv does not stall on the DMA
        xgs0 = []
        for g in range(2):
            xg = e.xp.tile([128, 4, 2, 132, 4], FP8, name=f"xgp{g}",
                           tag="xg")
            nc.sync.dma_start(out=xg[:], in_=e.io["x"][g])
            xgs0.append(xg)
    _phase0(e)
    tmp2_cur = _conv_stage(e, 0, xgs=xgs0)
    if first:
        _late_weights(e)
    for sg in range(4):
        qk_sb, ck_sb, pows = _proj_stage(e, tmp2_cur)
        tmp2_next = _conv_stage(e, sg + 1) if sg < 3 else None
        ar = _attn_stage(e, sg, tmp2_cur, qk_sb, ck_sb, pows)
        _deconv_score_stage(e, sg, ar)
        tmp2_cur = tmp2_next


def _emit(ctx, tc, io, reps=1, loop_reps=0):
    e = _setup(ctx, tc, io)
    if loop_reps:
        # timing-only variant: body wrapped in a hardware loop so device
        # time dominates RPC noise; late weights hoisted before the loop
        _late_weights(e)
        with tc.For_i(0, loop_reps):
            _body(e, first=False)
        return
    for _rep in range(reps):
        _body(e, first=(_rep == 0))


# ---------------------------------------------------------------------------
# program build (cached)
# ---------------------------------------------------------------------------

_CACHE = {}


def _build(reps=1, loop_reps=0):
    key = ("nc", reps, loop_reps)
    if key in _CACHE:
        return _CACHE[key]
    nc = bacc.Bacc("TRN2", target_bir_lowering=False, debug=False)
    d = {}

    def di(name, shape, dt):
        d[name] = nc.dram_tensor(name, shape, dt, kind="ExternalInput").ap()

    di("x", [8, 128, 4, 2, 132, 4], FP8)
    di("c1w", [128, 4, 2, 5, C1], FP8)
    di("c2w", [128, 2, 2, 5, C2], FP8)
    di("dc1w", [128, 4, 2, 4, D1], FP8)
    di("dc2w", [128, 2, 2, 4, D2], FP8)
    di("saq", [128, 2, C2], BF16)
    di("sak", [128, 2, C2], BF16)
    di("cak", [128, 2, C2], BF16)
    di("caq", [128, 3, C2], BF16)
    di("s1w", [128, SIM], BF16)
    di("s2w", [128, 3, SIM], BF16)
    di("c1b", [128, 4], F32)
    di("c2b", [128, 2], F32)
    di("dc1b", [128, 4], F32)
    di("dc2b", [128, 1], F32)
    di("sbqk", [128, 2], F32)
    di("cab", [128, 2], F32)
    di("mlpw", [128, 1], F32)
    di("mlpb", [16, 1], F32)
    di("cvec", [128, 3, 2], BF16)
    di("ve21", [128, 2], F32)
    di("ve12", [128, 2], F32)
    di("vch", [128, 2], F32)
    di("cve21", [128, 2], F32)
    di("cve12", [128, 2], F32)
    di("cavch", [128, 2], F32)
    di("blk", [128, 4, 2], BF16)
    di("mb4", [128, 4, 2, 128], F32)
    di("mc4", [2, 4, 2, 128], F32)
    d["out"] = nc.dram_tensor("out", [2, SEG, 128], F32,
                              kind="ExternalOutput").ap()
    with tile.TileContext(nc) as tc:
        with ExitStack() as ctx:
            _emit(ctx, tc, d, reps=reps, loop_reps=loop_reps)
    nc.compile()
    _CACHE[key] = nc
    return nc


# ---------------------------------------------------------------------------
# host-side prep (layout/cast only)
# ---------------------------------------------------------------------------

def _chunk_bias(v, nchunk):
    return np.ascontiguousarray(
        np.asarray(v, np.float32).reshape(nchunk, 128).T)


def _band(v, dtype=None, scale=1.0):
    # [128, 2, 256]; column 128 of chunk ht = v[ht*128:(ht+1)*128]
    dtype = dtype or nbf
    out = np.zeros((128, 2, 256), dtype)
    vv = np.asarray(v, np.float32).reshape(2, 128).T * scale
    out[:, :, 128] = vv.astype(dtype)
    return out


def _wchunks(w, nk, dtype=None, scale=1.0):
    # w: [K, ...] -> [128, nk, ...] (zero-pad K up to nk*128)
    w = np.asarray(w, np.float32) * scale
    k = w.shape[0]
    if k < nk * 128:
        w = np.concatenate(
            [w, np.zeros((nk * 128 - k,) + w.shape[1:], np.float32)], 0)
    w = w.reshape((nk, 128) + w.shape[1:])
    perm = (1, 0) + tuple(range(2, w.ndim))
    return np.ascontiguousarray(w.transpose(perm)).astype(dtype or nbf)


def prepare_common(inp):
    g = {}
    g["c1w"] = _wchunks(np.asarray(inp["conv1_w"], np.float32)
                        .transpose(1, 2, 0), 8, nf8, SC).reshape(
                            128, 4, 2, 5, C1)
    g["c2w"] = _wchunks(np.asarray(inp["conv2_w"], np.float32)
                        .transpose(1, 2, 0), 4, nf8, SC).reshape(
                            128, 2, 2, 5, C2)
    g["dc1w"] = _wchunks(np.asarray(inp["dc1_w"], np.float32)
                         .transpose(0, 2, 1), 8, nf8, SC).reshape(
                             128, 4, 2, 4, D1)
    g["dc2w"] = _wchunks(np.asarray(inp["dc2_w"], np.float32)
                         .transpose(0, 2, 1), 4, nf8, SC).reshape(
                             128, 2, 2, 4, D2)
    g["saq"] = _wchunks(inp["sa_wq"], 2)
    g["sak"] = _wchunks(inp["sa_wk"], 2)
    g["cak"] = _wchunks(inp["ca_wk"], 2)
    g["caq"] = _wchunks(inp["ca_wq"], 3)              # [128,3,256]
    g["s1w"] = np.ascontiguousarray(
        np.asarray(inp["sim1_w"], np.float32)).astype(nbf)
    g["s2w"] = _wchunks(inp["sim2_w"], 3)             # [128,3,128]
    vv = np.ascontiguousarray(
        np.asarray(inp["sa_v"], np.float32).reshape(2, 128).T)
    g["ve21"] = PE3 * vv
    g["ve12"] = PE3 * vv
    g["vch"] = vv
    cvv = np.ascontiguousarray(
        np.asarray(inp["ca_v"], np.float32).reshape(2, 128).T)
    g["cve21"] = PE3 * cvv
    g["cve12"] = PE3 * cvv
    g["cavch"] = cvv
    blk = np.zeros((128, 4, 2), np.float32)
    for s in range(4):
        blk[s * 32 : (s + 1) * 32, s, :] = 1.0
    g["blk"] = blk.astype(nbf)
    g["c1b"] = _chunk_bias(inp["conv1_b"], 4)
    g["c2b"] = _chunk_bias(inp["conv2_b"], 2)
    g["dc1b"] = _chunk_bias(inp["dc1_b"], 4)
    g["dc2b"] = _chunk_bias(inp["dc2_b"], 1)
    g["sbqk"] = _chunk_bias(
        np.asarray(inp["sa_bq"], np.float32)
        + np.asarray(inp["sa_bk"], np.float32), 2)
    g["cab"] = _chunk_bias(
        np.asarray(inp["ca_bq"], np.float32)
        + np.asarray(inp["ca_bk"], np.float32), 2)
    g["mlpw"] = np.ascontiguousarray(
        np.asarray(inp["mlp_w"], np.float32).reshape(128, 1))
    g["mlpb"] = np.full((16, 1), float(np.asarray(inp["mlp_b"])) * 0.5,
                        np.float32)
    return g


def prepare_core(inp, b):
    o = {}
    x = np.asarray(inp["batch"], np.float32)[b]       # [32,128,1024]
    x = x.transpose(0, 2, 1)                          # [32,1024,128]
    xp = np.zeros((SEG, CIN, 132), np.float32)
    xp[:, :, 2:130] = x
    xp = xp.reshape(8, 4, 8, 128, 132).transpose(0, 3, 2, 4, 1)
    o["x"] = np.ascontiguousarray(xp).astype(nf8).reshape(
        8, 128, 4, 2, 132, 4)    # [grp, p, kcp, two, pos, seg] seg-minor
    cv = np.zeros((2, 384), np.float32)
    cv[0, :CD] = np.asarray(inp["concept1"], np.float32)[b]
    cv[1, :CD] = np.asarray(inp["concept2"], np.float32)[b]
    o["cvec"] = np.ascontiguousarray(
        cv.reshape(2, 3, 128).transpose(2, 1, 0)).astype(nbf)  # [128,3,2]
    sl = np.asarray(inp["seg_len"], np.int64)[b]      # [32]
    k = np.arange(L4)
    mrow = np.where(sl[:, None] > 4 * k[None, :], 0.0, NEG).astype(np.float32)
    mb4 = np.full((128, 4, 2, 128), NEG, np.float32)
    mc4 = np.empty((2, 4, 2, 128), np.float32)
    for sg in range(4):
        for g in range(2):
            base = sg * 8 + g * 4
            for s in range(4):
                mb4[s * 32 : (s + 1) * 32, sg, g, s * 32 : (s + 1) * 32] = (
                    mrow[base + s][None, :])
            mc4[:, sg, g, :] = mrow[base : base + 4].reshape(1, 128)
    o["mb4"] = mb4
    o["mc4"] = mc4
    return o


def kernel(**inputs):
    nc = _build()
    common = prepare_common(inputs)
    in_maps = []
    for b in range(B):
        m = dict(common)
        m.update(prepare_core(inputs, b))
        in_maps.append(m)
    res = run_bass_kernel_spmd(nc, in_maps, list(range(B)))
    s1 = np.stack([res.results[b]["out"][0] for b in range(B)])
    s2 = np.stack([res.results[b]["out"][1] for b in range(B)])
    return s1.astype(np.float32), s2.astype(np.float32)

